# revision 7
# baseline (speedup 1.0000x reference)
"""Trainium2 Bass kernel for nn_MixtureOfExperts (8 experts, 8 cores).

Strategy (expert-parallel + batch-parallel GS):
  - Core e holds expert e's weights (fp16) and computes the 3-GEMM expert MLP
    for the FULL batch in a transposed chain:
        h1T = relu(W1e^T @ xT + b1)   [H1, B]
        h2T = relu(W2e^T @ h1T + b2)  [H2, B]
        y   = h2T^T @ W3e + b3        [B, OUT]   (lhsT = h2T slice, rhs = W3e)
    so weights are always the stationary operand in their natural layout and
    the only transpose (x -> xT) is done on the host.
  - AllToAll redistributes y: core e ends with all 8 experts' outputs for its
    512-sample batch slice (uniform SPMD indexing; collective blocks = slices).
  - Per-sample Gram-Schmidt over the 8 expert vectors (D=512) as classical GS
    on UNnormalized vectors (mathematically identical to the reference's
    modified GS since the v_j are orthogonal):
        d_ij = u_i . v_j            (fused DVE dot: tensor_tensor_reduce)
        v_i  = u_i - sum_j d_ij/(n_j^2 (1+eps)) v_j
    normalization is folded into per-sample scalars; the combine+Wo dot
    reduces to final = sum_i gw_i * (v_i . Wo) / n_i + bo.
  - Gating MLP + softmax computed per core for its slice only.
"""

import numpy as np

import concourse.bass as bass
import concourse.mybir as mybir
import concourse.tile as tile
from concourse import bacc
from concourse.bass_utils import run_bass_kernel_spmd

F16 = mybir.dt.float16
F32 = mybir.dt.float32
AF = mybir.ActivationFunctionType
OP = mybir.AluOpType

B, IN, H1, H2, OUT, E, G = 4096, 1024, 2048, 2048, 512, 8, 256
P = 128
EPS = 1e-6
NCORES = 8
BSL = B // NCORES          # per-core batch slice for GS (512)
NB = 512                   # GEMM batch-chunk (moving free dim)
NCHUNK = B // NB           # 8
K1, K2, K3 = IN // P, H1 // P, H2 // P   # 8, 16, 16
M1, M2 = H1 // P, H2 // P                # 16, 16
GK = G // P                # 2
NBB = BSL // P             # 4 GS sub-chunks per core


def build_nc():
    nc = bacc.Bacc("TRN2", target_bir_lowering=False, debug=False,
                   num_devices=NCORES)

    # ---- DRAM I/O ----
    xT = nc.dram_tensor("xT", [P, K1, B], F16, kind="ExternalInput")
    w1 = nc.dram_tensor("w1", [P, K1, H1], F16, kind="ExternalInput")
    w2 = nc.dram_tensor("w2", [P, K2, H2], F16, kind="ExternalInput")
    w3 = nc.dram_tensor("w3", [P, K3, OUT], F16, kind="ExternalInput")
    b1 = nc.dram_tensor("b1", [P, M1], F32, kind="ExternalInput")
    b2 = nc.dram_tensor("b2", [P, M2], F32, kind="ExternalInput")
    b3b = nc.dram_tensor("b3b", [P, OUT], F32, kind="ExternalInput")
    xg = nc.dram_tensor("xg", [P, K1, BSL], F16, kind="ExternalInput")
    wg1 = nc.dram_tensor("wg1", [P, K1, G], F16, kind="ExternalInput")
    bg1 = nc.dram_tensor("bg1", [P, GK], F32, kind="ExternalInput")
    wg2 = nc.dram_tensor("wg2", [P, GK, E], F16, kind="ExternalInput")
    bg2b = nc.dram_tensor("bg2b", [P, E], F32, kind="ExternalInput")
    wob = nc.dram_tensor("wob", [P, OUT], F32, kind="ExternalInput")
    bob = nc.dram_tensor("bob", [P, 1], F32, kind="ExternalInput")
    out = nc.dram_tensor("out", [BSL, 1], F32, kind="ExternalOutput")

    with tile.TileContext(nc) as tc:
        with (
            tc.tile_pool(name="weights", bufs=1) as wp,
            tc.tile_pool(name="dram", bufs=1, space="DRAM") as dp,
        ):
            w1_sb = wp.tile([P, K1, H1], F16)
            w2_sb = wp.tile([P, K2, H2], F16)
            w3_sb = wp.tile([P, K3, OUT], F16)
            b1_sb = wp.tile([P, M1], F32)
            b2_sb = wp.tile([P, M2], F32)
            b3b_sb = wp.tile([P, OUT], F32)
            xg_sb = wp.tile([P, K1, BSL], F16)
            wg1_sb = wp.tile([P, K1, G], F16)
            bg1_sb = wp.tile([P, GK], F32)
            wg2_sb = wp.tile([P, GK, E], F16)
            bg2b_sb = wp.tile([P, E], F32)
            wob_sb = wp.tile([P, OUT], F32)
            bob_sb = wp.tile([P, 1], F32)
            for sb_t, dr_t in [
                (w1_sb, w1), (w2_sb, w2), (w3_sb, w3), (b1_sb, b1),
                (b2_sb, b2), (b3b_sb, b3b), (xg_sb, xg), (wg1_sb, wg1),
                (bg1_sb, bg1), (wg2_sb, wg2), (bg2b_sb, bg2b),
                (wob_sb, wob), (bob_sb, bob),
            ]:
                nc.sync.dma_start(sb_t[:], dr_t.ap())

            y_dram = dp.tile([B, OUT], F16)      # AllToAll input bounce
            u_dram = dp.tile([B, OUT], F16)      # AllToAll output bounce

            # ---------------- Phase 1: expert GEMMs ----------------
            with (
                tc.tile_pool(name="gemm_sb", bufs=2) as gp,
                tc.tile_pool(name="gemm_ps", bufs=4, space="PSUM") as pp,
            ):
                for c in range(NCHUNK):
                    xt = gp.tile([P, K1, NB], F16, name="xt")
                    nc.sync.dma_start(
                        xt[:], xT.ap()[:, :, c * NB:(c + 1) * NB])

                    h1t = gp.tile([P, M1, NB], F16, name="h1t", bufs=1)
                    for m in range(M1):
                        ps = pp.tile([P, NB], F32, name="ps_g1", tag="ps")
                        for k in range(K1):
                            nc.tensor.matmul(
                                ps[:], w1_sb[:, k, m * P:(m + 1) * P],
                                xt[:, k, :],
                                start=(k == 0), stop=(k == K1 - 1))
                        nc.scalar.activation(
                            h1t[:, m, :], ps[:], AF.Relu,
                            bias=b1_sb[:, m:m + 1], scale=1.0)

                    h2t = gp.tile([P, M2, NB], F16, name="h2t", bufs=1)
                    for m in range(M2):
                        ps = pp.tile([P, NB], F32, name="ps_g2", tag="ps")
                        for k in range(K2):
                            nc.tensor.matmul(
                                ps[:], w2_sb[:, k, m * P:(m + 1) * P],
                                h1t[:, k, :],
                                start=(k == 0), stop=(k == K2 - 1))
                        nc.scalar.activation(
                            h2t[:, m, :], ps[:], AF.Relu,
                            bias=b2_sb[:, m:m + 1], scale=1.0)

                    for bb in range(NB // P):
                        ps = pp.tile([P, OUT], F32, name="ps_g3", tag="ps")
                        for k in range(K3):
                            nc.tensor.matmul(
                                ps[:], h2t[:, k, bb * P:(bb + 1) * P],
                                w3_sb[:, k, :],
                                start=(k == 0), stop=(k == K3 - 1))
                        y_sb = gp.tile([P, OUT], F16, name="y_sb")
                        nc.vector.tensor_add(y_sb[:], ps[:], b3b_sb[:])
                        r0 = c * NB + bb * P
                        nc.sync.dma_start(y_dram[r0:r0 + P, :], y_sb[:])

            # ---------------- AllToAll ----------------
            # y_dram block m (rows [m*512,(m+1)*512)) -> core m; received
            # block f = expert f's outputs for THIS core's batch slice.
            nc.gpsimd.collective_compute(
                "AllToAll", OP.bypass,
                replica_groups=[list(range(NCORES))],
                ins=[y_dram.opt()], outs=[u_dram.opt()])

            # ---------------- Phase 2+3: gating + Gram-Schmidt ----------------
            with (
                tc.tile_pool(name="gs_sb", bufs=2) as sp,
                tc.tile_pool(name="gs_small", bufs=2) as mp,
                tc.tile_pool(name="gs_ps", bufs=2, space="PSUM") as zp,
            ):
                # gating hidden layer for this core's 512 samples
                g_sb = mp.tile([P, GK, BSL], F16, bufs=1)
                for g in range(GK):
                    ps = zp.tile([P, BSL], F32, name="ps_gate")
                    for k in range(K1):
                        nc.tensor.matmul(
                            ps[:], wg1_sb[:, k, g * P:(g + 1) * P],
                            xg_sb[:, k, :],
                            start=(k == 0), stop=(k == K1 - 1))
                    nc.scalar.activation(
                        g_sb[:, g, :], ps[:], AF.Relu,
                        bias=bg1_sb[:, g:g + 1], scale=1.0)

                for bb in range(NBB):
                    # --- gate logits + softmax for these 128 samples ---
                    zps = zp.tile([P, E], F32, name="ps_z")
                    for g in range(GK):
                        nc.tensor.matmul(
                            zps[:], g_sb[:, g, bb * P:(bb + 1) * P],
                            wg2_sb[:, g, :],
                            start=(g == 0), stop=(g == GK - 1))
                    z_sb = mp.tile([P, E], F32, name="z_sb")
                    nc.vector.tensor_add(z_sb[:], zps[:], bg2b_sb[:])
                    mx = mp.tile([P, 1], F32, name="mx")
                    nc.vector.reduce_max(mx[:], z_sb[:], axis=mybir.AxisListType.X)
                    nmx = mp.tile([P, 1], F32, name="nmx")
                    nc.vector.tensor_scalar_mul(nmx[:], mx[:], -1.0)
                    ez = mp.tile([P, E], F32, name="ez")
                    sez = mp.tile([P, 1], F32, name="sez")
                    nc.scalar.activation(ez[:], z_sb[:], AF.Exp,
                                         bias=nmx[:], scale=1.0,
                                         accum_out=sez[:])
                    rsez = mp.tile([P, 1], F32, name="rsez")
                    nc.vector.reciprocal(rsez[:], sez[:])
                    gw = mp.tile([P, E], F32, name="gw")
                    nc.vector.tensor_scalar_mul(gw[:], ez[:], rsez[:])

                    # --- load the 8 expert vectors for these samples ---
                    u_sb = sp.tile([P, E, OUT], F16, name="u_sb")
                    for f in range(E):
                        r0 = f * BSL + bb * P
                        nc.sync.dma_start(u_sb[:, f, :], u_dram[r0:r0 + P, :])

                    # --- classical GS on unnormalized v ---
                    v_sb = sp.tile([P, E, OUT], F32, name="v_sb", bufs=1)
                    nsq = mp.tile([P, E], F32, name="nsq")
                    ninv = mp.tile([P, E], F32, name="ninv")
                    d = mp.tile([P, E], F32, name="d")
                    s = mp.tile([P, E], F32, name="s")
                    q = mp.tile([P, E], F32, name="q")

                    def scr():
                        return mp.tile([P, OUT], F32, name="scr")

                    nc.scalar.copy(v_sb[:, 0, :], u_sb[:, 0, :])
                    # nsq_0 = sum(u_0^2) on ACT (Square + accum)
                    nc.scalar.activation(scr()[:], v_sb[:, 0, :], AF.Square,
                                         accum_out=nsq[:, 0:1])
                    # ninv_0 = -1/(nsq_0*(1+eps))
                    t0 = mp.tile([P, 1], F32, name="t0")
                    nc.vector.tensor_scalar_mul(t0[:], nsq[:, 0:1], -(1.0 + EPS))
                    nc.vector.reciprocal(ninv[:, 0:1], t0[:])

                    for i in range(1, E):
                        for j in range(i):
                            nc.vector.scalar_tensor_tensor(
                                out=scr()[:], in0=u_sb[:, i, :], scalar=1.0,
                                in1=v_sb[:, j, :], op0=OP.mult, op1=OP.mult,
                                accum_out=d[:, j:j + 1])
                        # s_j = d_j * ninv_j   (== -coeff_ij)
                        nc.vector.tensor_mul(s[:, :i], d[:, :i], ninv[:, :i])
                        # v_i = u_i + sum_j s_j v_j
                        nc.vector.scalar_tensor_tensor(
                            out=v_sb[:, i, :], in0=v_sb[:, 0, :],
                            scalar=s[:, 0:1], in1=u_sb[:, i, :],
                            op0=OP.mult, op1=OP.add)
                        for j in range(1, i):
                            nc.vector.scalar_tensor_tensor(
                                out=v_sb[:, i, :], in0=v_sb[:, j, :],
                                scalar=s[:, j:j + 1], in1=v_sb[:, i, :],
                                op0=OP.mult, op1=OP.add)
                        nc.scalar.activation(scr()[:], v_sb[:, i, :], AF.Square,
                                             accum_out=nsq[:, i:i + 1])
                        ti = mp.tile([P, 1], F32, name="ti")
                        nc.vector.tensor_scalar_mul(ti[:], nsq[:, i:i + 1],
                                                    -(1.0 + EPS))
                        nc.vector.reciprocal(ninv[:, i:i + 1], ti[:])

                    # norms: n_i = max(sqrt(nsq_i), eps); inn = 1/n
                    nrm = mp.tile([P, E], F32, name="nrm")
                    nc.scalar.sqrt(nrm[:], nsq[:])
                    nc.vector.tensor_scalar_max(nrm[:], nrm[:], EPS)
                    inn = mp.tile([P, E], F32, name="inn")
                    nc.vector.reciprocal(inn[:], nrm[:])

                    # q_i = v_i . Wo
                    for i in range(E):
                        nc.vector.scalar_tensor_tensor(
                            out=scr()[:], in0=v_sb[:, i, :], scalar=1.0,
                            in1=wob_sb[:], op0=OP.mult, op1=OP.mult,
                            accum_out=q[:, i:i + 1])

                    # final = sum_i gw_i * q_i * inn_i + bo
                    t1 = mp.tile([P, E], F32, name="t1")
                    nc.vector.tensor_mul(t1[:], q[:], inn[:])
                    nc.vector.tensor_mul(t1[:], t1[:], gw[:])
                    fin = mp.tile([P, 1], F32, name="fin")
                    nc.vector.reduce_sum(fin[:], t1[:], axis=mybir.AxisListType.X)
                    nc.vector.tensor_add(fin[:], fin[:], bob_sb[:])
                    nc.sync.dma_start(out.ap()[bb * P:(bb + 1) * P, :], fin[:])

    nc.compile()
    return nc


_NC = None
_last_in_maps = None


def _get_nc():
    global _NC
    if _NC is None:
        _NC = build_nc()
    return _NC


def _tile_k(w):
    """[K*128, M] -> [128, K, M] with t[p, ko, m] = w[ko*128+p, m]."""
    Kp, M = w.shape
    return np.ascontiguousarray(
        w.reshape(Kp // 128, 128, M).transpose(1, 0, 2))


def kernel(x, W1, b1, W2, b2, W3, b3, Wg1, bg1, Wg2, bg2, Wo, bo):
    x = np.asarray(x, dtype=np.float32)
    xT = _tile_k(np.ascontiguousarray(x.T)).astype(np.float16)  # [128,8,B]
    wg1_t = _tile_k(np.asarray(Wg1, np.float32)).astype(np.float16)
    bg1_t = np.ascontiguousarray(
        np.asarray(bg1, np.float32).reshape(GK, P).T)
    wg2_t = _tile_k(np.asarray(Wg2, np.float32)).astype(np.float16)
    bg2b = np.ascontiguousarray(
        np.broadcast_to(np.asarray(bg2, np.float32), (P, E)))
    wob = np.ascontiguousarray(
        np.broadcast_to(np.asarray(Wo, np.float32)[:, 0], (P, OUT)))
    bob = np.ascontiguousarray(
        np.broadcast_to(np.asarray(bo, np.float32), (P, 1)))

    in_maps = []
    for e in range(NCORES):
        in_maps.append({
            "xT": xT,
            "w1": _tile_k(np.asarray(W1[e], np.float32)).astype(np.float16),
            "w2": _tile_k(np.asarray(W2[e], np.float32)).astype(np.float16),
            "w3": _tile_k(np.asarray(W3[e], np.float32)).astype(np.float16),
            "b1": np.ascontiguousarray(
                np.asarray(b1[e], np.float32).reshape(M1, P).T),
            "b2": np.ascontiguousarray(
                np.asarray(b2[e], np.float32).reshape(M2, P).T),
            "b3b": np.ascontiguousarray(
                np.broadcast_to(np.asarray(b3[e], np.float32), (P, OUT))),
            "xg": np.ascontiguousarray(xT[:, :, e * BSL:(e + 1) * BSL]),
            "wg1": wg1_t,
            "bg1": bg1_t,
            "wg2": wg2_t,
            "bg2b": bg2b,
            "wob": wob,
            "bob": bob,
        })

    global _last_in_maps
    _last_in_maps = in_maps
    nc = _get_nc()
    res = run_bass_kernel_spmd(nc, in_maps, core_ids=list(range(NCORES)))
    final = np.concatenate(
        [res.results[c]["out"] for c in range(NCORES)], axis=0)
    return (final.astype(np.float32), 0.0)


# revision 8
# speedup vs baseline: 1.1724x; 1.1724x over previous
"""Trainium2 Bass kernel for nn_MixtureOfExperts (8 experts, 8 cores).

v2: expert-parallel GEMMs + quarter-split AllToAll + software-pipelined
Gram-Schmidt overlapped with the GEMM stream.

  - Core e holds expert e's weights (fp16) and computes the 3-GEMM expert MLP
    for the FULL batch in a transposed chain (weights stationary, natural
    layout; only x is transposed, on the host):
        h1T = relu(W1e^T @ xT + b1)   [H1, B]
        h2T = relu(W2e^T @ h1T + b2)  [H2, B]
        y   = h2T^T @ W3e + b3        [B, OUT]
    Batch is processed in 16 chunks of 256 columns.
  - After each group of 4 chunks (1024 batch rows) an AllToAll redistributes
    that quarter of y: core e receives all 8 experts' vectors for samples
    [q*1024 + e*128, q*1024 + (e+1)*128).
  - Gram-Schmidt (classical GS on unnormalized vectors, normalization folded
    into per-sample scalars; mathematically = reference's modified GS) runs
    on the vector engine, emitted 1-2 chunks AFTER its AllToAll so the
    in-order engine queues never stall the PE: DVE work one chunk late,
    the tiny ACT work (sqrt) two chunks late. u/y/out DMAs ride the gpsimd
    SWDGE queue; xt/weight DMAs the sync HWDGE queue.
  - Gating MLP + softmax per core for its own 512 samples (input xg is the
    host-sliced, quarter-ordered transpose of x).
"""

import numpy as np

import concourse.bass as bass
import concourse.mybir as mybir
import concourse.tile as tile
from concourse import bacc
from concourse.bass_utils import run_bass_kernel_spmd

F16 = mybir.dt.float16
F32 = mybir.dt.float32
AF = mybir.ActivationFunctionType
OP = mybir.AluOpType

B, IN, H1, H2, OUT, E, G = 4096, 1024, 2048, 2048, 512, 8, 256
P = 128
EPS = 1e-6
NCORES = 8
BSL = B // NCORES          # 512 samples per core for GS
NB = 256                   # GEMM batch-chunk (moving free dim)
NCHUNK = B // NB           # 16
NQ = 4                     # AllToAll quarters
QROWS = B // NQ            # 1024
K1, K2, K3 = IN // P, H1 // P, H2 // P   # 8, 16, 16
M1, M2 = H1 // P, H2 // P                # 16, 16
GK = G // P                # 2


def build_nc():
    nc = bacc.Bacc("TRN2", target_bir_lowering=False, debug=False,
                   num_devices=NCORES)

    xT = nc.dram_tensor("xT", [P, K1, B], F16, kind="ExternalInput")
    w1 = nc.dram_tensor("w1", [P, K1, H1], F16, kind="ExternalInput")
    w2 = nc.dram_tensor("w2", [P, K2, H2], F16, kind="ExternalInput")
    w3 = nc.dram_tensor("w3", [P, K3, OUT], F16, kind="ExternalInput")
    b1 = nc.dram_tensor("b1", [P, M1], F32, kind="ExternalInput")
    b2 = nc.dram_tensor("b2", [P, M2], F32, kind="ExternalInput")
    b3b = nc.dram_tensor("b3b", [P, OUT], F32, kind="ExternalInput")
    xg = nc.dram_tensor("xg", [P, K1, BSL], F16, kind="ExternalInput")
    wg1 = nc.dram_tensor("wg1", [P, K1, G], F16, kind="ExternalInput")
    bg1 = nc.dram_tensor("bg1", [P, GK], F32, kind="ExternalInput")
    wg2 = nc.dram_tensor("wg2", [P, GK, E], F16, kind="ExternalInput")
    bg2b = nc.dram_tensor("bg2b", [P, E], F32, kind="ExternalInput")
    wob = nc.dram_tensor("wob", [P, OUT], F32, kind="ExternalInput")
    bob = nc.dram_tensor("bob", [P, 1], F32, kind="ExternalInput")
    out = nc.dram_tensor("out", [BSL, 1], F32, kind="ExternalOutput")

    with tile.TileContext(nc) as tc:
        with (
            tc.tile_pool(name="weights", bufs=1) as wp,
            tc.tile_pool(name="dram", bufs=1, space="DRAM") as dp,
            tc.tile_pool(name="gemm_sb", bufs=2) as gp,
            tc.tile_pool(name="gemm_ps", bufs=4, space="PSUM") as pp,
            tc.tile_pool(name="gs_sb", bufs=1) as sp,
            tc.tile_pool(name="gs_small", bufs=2) as mp,
            tc.tile_pool(name="gs_ps", bufs=2, space="PSUM") as zp,
        ):
            # ---- weight/const loads; w1 + first xt first so PE starts early
            w1_sb = wp.tile([P, K1, H1], F16)
            nc.sync.dma_start(w1_sb[:], w1.ap())
            xt0 = gp.tile([P, K1, NB], F16, name="xt", tag="xt")
            nc.sync.dma_start(xt0[:], xT.ap()[:, :, 0:NB])

            w2_sb = wp.tile([P, K2, H2], F16)
            w3_sb = wp.tile([P, K3, OUT], F16)
            b1_sb = wp.tile([P, M1], F32)
            b2_sb = wp.tile([P, M2], F32)
            b3b_sb = wp.tile([P, OUT], F32)
            xg_sb = wp.tile([P, K1, BSL], F16)
            wg1_sb = wp.tile([P, K1, G], F16)
            bg1_sb = wp.tile([P, GK], F32)
            wg2_sb = wp.tile([P, GK, E], F16)
            bg2b_sb = wp.tile([P, E], F32)
            wob_sb = wp.tile([P, OUT], F32)
            bob_sb = wp.tile([P, 1], F32)
            for sb_t, dr_t in [
                (w2_sb, w2), (w3_sb, w3), (b1_sb, b1), (b2_sb, b2),
                (b3b_sb, b3b), (xg_sb, xg), (wg1_sb, wg1), (bg1_sb, bg1),
                (wg2_sb, wg2), (bg2b_sb, bg2b), (wob_sb, wob), (bob_sb, bob),
            ]:
                nc.sync.dma_start(sb_t[:], dr_t.ap())

            y_q = [dp.tile([QROWS, OUT], F16, name=f"y_q{q}")
                   for q in range(NQ)]
            u_q = [dp.tile([QROWS, OUT], F16, name=f"u_q{q}")
                   for q in range(NQ)]

            g_sb = sp.tile([P, GK, BSL], F16)

            # per-quarter GS state handed from stage1 to stage2
            gs_state = {}

            def emit_gating_hidden():
                for g in range(GK):
                    ps = zp.tile([P, BSL], F32, name="ps_gate", tag="zps")
                    for k in range(K1):
                        nc.tensor.matmul(
                            ps[:], wg1_sb[:, k, g * P:(g + 1) * P],
                            xg_sb[:, k, :],
                            start=(k == 0), stop=(k == K1 - 1))
                    nc.scalar.activation(
                        g_sb[:, g, :], ps[:], AF.Relu,
                        bias=bg1_sb[:, g:g + 1], scale=1.0)

            def emit_gs_stage1(q):
                """Softmax + all DVE Gram-Schmidt work for quarter q."""
                st = {}
                gs_state[q] = st
                # gate logits + softmax for samples [q*128:(q+1)*128) of xg
                zps = zp.tile([P, E], F32, name="ps_z", tag="zps")
                for g in range(GK):
                    nc.tensor.matmul(
                        zps[:], g_sb[:, g, q * P:(q + 1) * P],
                        wg2_sb[:, g, :],
                        start=(g == 0), stop=(g == GK - 1))
                z_sb = mp.tile([P, E], F32, name="z_sb")
                nc.vector.tensor_add(z_sb[:], zps[:], bg2b_sb[:])
                mx = mp.tile([P, 1], F32, name="mx")
                nc.vector.reduce_max(mx[:], z_sb[:], axis=mybir.AxisListType.X)
                nmx = mp.tile([P, 1], F32, name="nmx")
                nc.vector.tensor_scalar_mul(nmx[:], mx[:], -1.0)
                ez = mp.tile([P, E], F32, name="ez")
                sez = mp.tile([P, 1], F32, name="sez")
                nc.scalar.activation(ez[:], z_sb[:], AF.Exp,
                                     bias=nmx[:], scale=1.0, accum_out=sez[:])
                rsez = mp.tile([P, 1], F32, name="rsez")
                nc.vector.reciprocal(rsez[:], sez[:])
                gw = mp.tile([P, E], F32, name="gw", bufs=4)
                nc.vector.tensor_scalar_mul(gw[:], ez[:], rsez[:])
                st["gw"] = gw

                # expert vectors for this quarter (gpsimd DMA queue)
                u_sb = sp.tile([P, E, OUT], F16, name="u_sb")
                for f in range(E):
                    nc.gpsimd.dma_start(
                        u_sb[:, f, :], u_q[q][f * P:(f + 1) * P, :])

                v_sb = sp.tile([P, E, OUT], F16, name="v_sb")
                nsq = mp.tile([P, E], F32, name="nsq", bufs=4)
                ninv = mp.tile([P, E], F32, name="ninv")
                d = mp.tile([P, E], F32, name="d")
                s = mp.tile([P, E], F32, name="s")
                q_t = mp.tile([P, E], F32, name="q_t", bufs=4)
                st["nsq"], st["q_t"] = nsq, q_t

                def scr():
                    return mp.tile([P, OUT], F16, name="scr")

                nc.vector.tensor_copy(v_sb[:, 0, :], u_sb[:, 0, :])
                nc.vector.scalar_tensor_tensor(
                    out=scr()[:], in0=v_sb[:, 0, :], scalar=1.0,
                    in1=v_sb[:, 0, :], op0=OP.mult, op1=OP.mult,
                    accum_out=nsq[:, 0:1])
                t0 = mp.tile([P, 1], F32, name="t0")
                nc.vector.tensor_scalar_mul(t0[:], nsq[:, 0:1], -(1.0 + EPS))
                nc.vector.reciprocal(ninv[:, 0:1], t0[:])

                for i in range(1, E):
                    for j in range(i):
                        nc.vector.scalar_tensor_tensor(
                            out=scr()[:], in0=u_sb[:, i, :], scalar=1.0,
                            in1=v_sb[:, j, :], op0=OP.mult, op1=OP.mult,
                            accum_out=d[:, j:j + 1])
                    nc.vector.tensor_mul(s[:, :i], d[:, :i], ninv[:, :i])
                    nc.vector.scalar_tensor_tensor(
                        out=v_sb[:, i, :], in0=v_sb[:, 0, :],
                        scalar=s[:, 0:1], in1=u_sb[:, i, :],
                        op0=OP.mult, op1=OP.add)
                    for j in range(1, i):
                        nc.vector.scalar_tensor_tensor(
                            out=v_sb[:, i, :], in0=v_sb[:, j, :],
                            scalar=s[:, j:j + 1], in1=v_sb[:, i, :],
                            op0=OP.mult, op1=OP.add)
                    nc.vector.scalar_tensor_tensor(
                        out=scr()[:], in0=v_sb[:, i, :], scalar=1.0,
                        in1=v_sb[:, i, :], op0=OP.mult, op1=OP.mult,
                        accum_out=nsq[:, i:i + 1])
                    ti = mp.tile([P, 1], F32, name="ti")
                    nc.vector.tensor_scalar_mul(ti[:], nsq[:, i:i + 1],
                                                -(1.0 + EPS))
                    nc.vector.reciprocal(ninv[:, i:i + 1], ti[:])

                # q_i = v_i . Wo
                for i in range(E):
                    nc.vector.scalar_tensor_tensor(
                        out=scr()[:], in0=v_sb[:, i, :], scalar=1.0,
                        in1=wob_sb[:], op0=OP.mult, op1=OP.mult,
                        accum_out=q_t[:, i:i + 1])

            def emit_gs_stage2(q):
                """Finalize quarter q: norms (ACT sqrt) + combine + output."""
                st = gs_state.pop(q)
                nsq, q_t, gw = st["nsq"], st["q_t"], st["gw"]
                nrm = mp.tile([P, E], F32, name="nrm")
                nc.scalar.sqrt(nrm[:], nsq[:])
                nc.vector.tensor_scalar_max(nrm[:], nrm[:], EPS)
                inn = mp.tile([P, E], F32, name="inn")
                nc.vector.reciprocal(inn[:], nrm[:])
                t1 = mp.tile([P, E], F32, name="t1")
                nc.vector.tensor_mul(t1[:], q_t[:], inn[:])
                nc.vector.tensor_mul(t1[:], t1[:], gw[:])
                fin = mp.tile([P, 1], F32, name="fin")
                nc.vector.reduce_sum(fin[:], t1[:], axis=mybir.AxisListType.X)
                nc.vector.tensor_add(fin[:], fin[:], bob_sb[:])
                nc.gpsimd.dma_start(out.ap()[q * P:(q + 1) * P, :], fin[:])

            # ---------------- main chunk loop ----------------
            for c in range(NCHUNK):
                if c == 0:
                    xt = xt0
                else:
                    xt = gp.tile([P, K1, NB], F16, name="xt", tag="xt")
                    nc.sync.dma_start(
                        xt[:], xT.ap()[:, :, c * NB:(c + 1) * NB])

                h1t = gp.tile([P, M1, NB], F16, name="h1t", bufs=1)
                for m in range(M1):
                    ps = pp.tile([P, NB], F32, name="ps_g1", tag="ps")
                    for k in range(K1):
                        nc.tensor.matmul(
                            ps[:], w1_sb[:, k, m * P:(m + 1) * P],
                            xt[:, k, :],
                            start=(k == 0), stop=(k == K1 - 1))
                    nc.scalar.activation(
                        h1t[:, m, :], ps[:], AF.Relu,
                        bias=b1_sb[:, m:m + 1], scale=1.0)

                h2t = gp.tile([P, M2, NB], F16, name="h2t", bufs=1)
                for m in range(M2):
                    ps = pp.tile([P, NB], F32, name="ps_g2", tag="ps")
                    for k in range(K2):
                        nc.tensor.matmul(
                            ps[:], w2_sb[:, k, m * P:(m + 1) * P],
                            h1t[:, k, :],
                            start=(k == 0), stop=(k == K2 - 1))
                    nc.scalar.activation(
                        h2t[:, m, :], ps[:], AF.Relu,
                        bias=b2_sb[:, m:m + 1], scale=1.0)

                q_idx, cc = divmod(c, NQ)
                for bb in range(NB // P):
                    ps = pp.tile([P, OUT], F32, name="ps_g3", tag="ps")
                    for k in range(K3):
                        nc.tensor.matmul(
                            ps[:], h2t[:, k, bb * P:(bb + 1) * P],
                            w3_sb[:, k, :],
                            start=(k == 0), stop=(k == K3 - 1))
                    y_sb = gp.tile([P, OUT], F16, name="y_sb")
                    nc.vector.tensor_add(y_sb[:], ps[:], b3b_sb[:])
                    r0 = cc * NB + bb * P
                    nc.gpsimd.dma_start(y_q[q_idx][r0:r0 + P, :], y_sb[:])

                if c == 0:
                    emit_gating_hidden()
                if cc == NQ - 1:
                    nc.gpsimd.collective_compute(
                        "AllToAll", OP.bypass,
                        replica_groups=[list(range(NCORES))],
                        ins=[y_q[q_idx].opt()], outs=[u_q[q_idx].opt()])
                if c >= 4 and (c - 4) % 4 == 0:
                    emit_gs_stage1((c - 4) // 4)
                if c >= 5 and (c - 5) % 4 == 0:
                    emit_gs_stage2((c - 5) // 4)

            emit_gs_stage1(NQ - 1)
            emit_gs_stage2(NQ - 1)

    nc.compile()
    return nc


_NC = None
_last_in_maps = None


def _get_nc():
    global _NC
    if _NC is None:
        _NC = build_nc()
    return _NC


def _tile_k(w):
    """[K*128, M] -> [128, K, M] with t[p, ko, m] = w[ko*128+p, m]."""
    Kp, M = w.shape
    return np.ascontiguousarray(
        w.reshape(Kp // 128, 128, M).transpose(1, 0, 2))


def kernel(x, W1, b1, W2, b2, W3, b3, Wg1, bg1, Wg2, bg2, Wo, bo):
    x = np.asarray(x, dtype=np.float32)
    xT = _tile_k(np.ascontiguousarray(x.T)).astype(np.float16)  # [128,8,B]
    wg1_t = _tile_k(np.asarray(Wg1, np.float32)).astype(np.float16)
    bg1_t = np.ascontiguousarray(
        np.asarray(bg1, np.float32).reshape(GK, P).T)
    wg2_t = _tile_k(np.asarray(Wg2, np.float32)).astype(np.float16)
    bg2b = np.ascontiguousarray(
        np.broadcast_to(np.asarray(bg2, np.float32), (P, E)))
    wob = np.ascontiguousarray(
        np.broadcast_to(np.asarray(Wo, np.float32)[:, 0], (P, OUT)))
    bob = np.ascontiguousarray(
        np.broadcast_to(np.asarray(bo, np.float32), (P, 1)))

    in_maps = []
    for e in range(NCORES):
        # gating input: this core's GS samples in quarter order
        xg_cols = [xT[:, :, q * QROWS + e * P: q * QROWS + (e + 1) * P]
                   for q in range(NQ)]
        in_maps.append({
            "xT": xT,
            "w1": _tile_k(np.asarray(W1[e], np.float32)).astype(np.float16),
            "w2": _tile_k(np.asarray(W2[e], np.float32)).astype(np.float16),
            "w3": _tile_k(np.asarray(W3[e], np.float32)).astype(np.float16),
            "b1": np.ascontiguousarray(
                np.asarray(b1[e], np.float32).reshape(M1, P).T),
            "b2": np.ascontiguousarray(
                np.asarray(b2[e], np.float32).reshape(M2, P).T),
            "b3b": np.ascontiguousarray(
                np.broadcast_to(np.asarray(b3[e], np.float32), (P, OUT))),
            "xg": np.ascontiguousarray(np.concatenate(xg_cols, axis=2)),
            "wg1": wg1_t,
            "bg1": bg1_t,
            "wg2": wg2_t,
            "bg2b": bg2b,
            "wob": wob,
            "bob": bob,
        })

    global _last_in_maps
    _last_in_maps = in_maps
    nc = _get_nc()
    res = run_bass_kernel_spmd(nc, in_maps, core_ids=list(range(NCORES)))

    final = np.empty((B, 1), np.float32)
    for e in range(NCORES):
        o = res.results[e]["out"]
        for q in range(NQ):
            final[q * QROWS + e * P: q * QROWS + (e + 1) * P] = \
                o[q * P:(q + 1) * P]
    return (final, 0.0)
